# revision 7
# baseline (speedup 1.0000x reference)
"""Trainium2 Bass kernel for the batched CA_event ODE-RHS problem.

Computes, for B = 8388608 independent systems (per batch element):
    u  = W0*(x+e_x-t0) + W1*(y+e_y-t1)
    R_s = 1/(0.004*s^2+0.1)            # 10*(1-hill(s))
    dx = (10-Rx)*(1+u) + 0.2*Ry - 1.1*x
    dy = (10-Ry) + 0.2*Rx - 1.1*y
    out = [dx, dy, -dx, -dy]           # shape [B, 4]

Memory-bound problem; all device I/O is fp16 (harness gate is
scale-relative 2e-2; this pipeline lands ~2.5e-3).  Work is balanced
across three engines so the DMA stream (~51us/core) is the bottleneck:

  ACT   : v=Sq(.0632*xy) ; qx,qy=arsqrt(v+.1) ; rx02=Sq(sqrt(.2)qx)
          (=0.2Rx) ; a=Sq(sqrt(.2)qy) (=0.2Ry) ; Ry=Sq(qy)
  GPSIMD: ws=W0+W1 ; s1=m0+m1 ; u1=s1+wst1          (fp16 TTs)
  DVE   : pqa=xy+exy (TT 2x) ; m=wt*pqa (TT) ; wst1=ws*(-t)+1 (ts 4x)
          rx10=5*rx02-10 ; g=rx10*u1 ; x11=1.1x ; ca=a-x11 ; dx=ca-g
          y11=1.1y-10 ; d1=rx02-y11 ; dy=d1-Ry ; ndxy=dxy^0x8000

scalar_tensor_tensor runs at 1x only (no fp16 2x uop) so every chain is
built from tensor_tensor (2x) + tensor_scalar (4x) instead.

Outputs are written as planes [dx|dy] and [-dx|-dy] per chunk; the host
restacks to [B, 4] (pure gather, no math).  Batch is split evenly
across 8 NeuronCores; per-core 1048576 elements viewed as [128, 8192].
"""

import sys

import numpy as np

try:
    import concourse  # noqa: F401
except ImportError:  # pragma: no cover - fallback for bare environments
    sys.path.insert(0, "/opt/trn_rl_repo")

B = 8388608
N_CORES = 8
P = 128
BC = B // N_CORES          # 1048576 elements per core
COLS = BC // P             # 8192 free-dim columns per core
F = 2048                   # tile columns per loop iteration
N_IT = COLS // F

_COMPILED = {}

# config knobs (overridable from test.py for A/B runs)
FAST_RECIP = False         # kept for test.py compat (unused)
HILL = "act"               # "act": both R via ACT splines; "recip": Rx on DVE
GP_OFFLOAD = True          # ws/s1/u1 on GpSimd (else DVE)

SQ_SCALE = 0.0632455532    # sqrt(0.004): Square(SQ_SCALE*s) = 0.004*s^2
P2_SCALE = 0.4472135955    # sqrt(0.2):   Square(P2_SCALE*q) = 0.2*q^2


def _build(t0: float, t1: float):
    """Trace + compile the per-core Tile kernel. Returns a ready Bass object."""
    from contextlib import ExitStack

    import concourse.bacc as bacc
    import concourse.tile as tile
    from concourse import mybir
    from concourse.dve_ops import (
        RECIP_APPROX_FAST_CONSTS,
        RECIPROCAL_APPROX_FAST,
    )

    f16 = mybir.dt.float16
    f32 = mybir.dt.float32
    i16 = mybir.dt.int16
    ADD = mybir.AluOpType.add
    SUB = mybir.AluOpType.subtract
    MUL = mybir.AluOpType.mult
    XOR = mybir.AluOpType.bitwise_xor
    SQUARE = mybir.ActivationFunctionType.Square
    ARSQRT = mybir.ActivationFunctionType.Abs_reciprocal_sqrt

    assert t0 == t1

    nc = bacc.Bacc("TRN2", target_bir_lowering=False, debug=False,
                   num_devices=N_CORES)

    # bias constant for the arsqrt activation (bias APs must pre-exist)
    _c = nc.alloc_sbuf_tensor("const-float32-0.1", [128, 1], f32)
    nc.gpsimd.memset(_c.ap(), 0.1)
    nc.const_aps.aps[(f32, 0.1)] = _c.ap()
    nc.all_engine_barrier()

    in_d = nc.dram_tensor("inp", [P, 6 * COLS], f16,
                          kind="ExternalInput").ap()
    o_d = nc.dram_tensor("out", [P, 4 * COLS], f16, kind="ExternalOutput").ap()

    ve = nc.vector
    gp = nc.gpsimd if GP_OFFLOAD else nc.vector

    with tile.TileContext(nc) as tc:
        with ExitStack() as ctx:
            io = ctx.enter_context(tc.tile_pool(name="io", bufs=2))
            tp = ctx.enter_context(tc.tile_pool(name="tmp", bufs=2))

            for c in range(N_IT):
                it = io.tile([P, 6 * F], f16, tag="in")
                dxy = io.tile([P, 2 * F], f16, tag="dxy")
                ndxy = io.tile([P, 2 * F], f16, tag="ndxy")

                # packed layout per chunk: [x|y|ex|ey|W0|W1], F cols each
                nc.sync.dma_start(it[:, :4 * F],
                                  in_d[:, 6 * F * c:6 * F * c + 4 * F])
                nc.sync.dma_start(it[:, 4 * F:],
                                  in_d[:, 6 * F * c + 4 * F:6 * F * (c + 1)])
                xy = it[:, :2 * F]
                exy = it[:, 2 * F:4 * F]
                wt = it[:, 4 * F:]

                pqa = tp.tile([P, 2 * F], f16, tag="pqa", bufs=1)
                m = tp.tile([P, 2 * F], f16, tag="m")
                ws = tp.tile([P, F], f16, tag="ws")
                wst1 = tp.tile([P, F], f16, tag="wst1")
                s1 = tp.tile([P, F], f16, tag="s1", bufs=1)
                u1 = tp.tile([P, F], f16, tag="u1")
                v = tp.tile([P, 2 * F], f16, tag="v", bufs=1)
                qx = tp.tile([P, F], f16, tag="qx", bufs=1)
                qy = tp.tile([P, F], f16, tag="qy", bufs=1)
                rx02 = tp.tile([P, F], f16, tag="rx02")
                aa = tp.tile([P, F], f16, tag="aa")
                ry = tp.tile([P, F], f16, tag="ry")
                rx10 = tp.tile([P, F], f16, tag="rx10", bufs=1)
                x11 = tp.tile([P, F], f16, tag="x11", bufs=1)
                y11 = tp.tile([P, F], f16, tag="y11", bufs=1)
                gg = tp.tile([P, F], f16, tag="gg", bufs=1)
                ca = tp.tile([P, F], f16, tag="ca", bufs=1)
                d1 = tp.tile([P, F], f16, tag="d1", bufs=1)

                # --- u path ---
                # pqa = xy+exy ; m = wt*pqa ; ws = W0+W1 ;
                # wst1 = -t*ws+1 ; s1 = m0+m1 ; u1 = s1+wst1  (= 1+u)
                ve.tensor_tensor(pqa[:], xy, exy, ADD)
                ve.tensor_tensor(m[:], wt, pqa[:], MUL)
                gp.tensor_tensor(ws[:], wt[:, :F], wt[:, F:], ADD)
                ve.tensor_scalar(wst1[:], ws[:], -t0, 1.0, MUL, ADD)
                gp.tensor_tensor(s1[:], m[:, :F], m[:, F:], ADD)
                gp.tensor_tensor(u1[:], s1[:], wst1[:], ADD)

                # --- Hill path (ACT) ---
                # v = 0.004*s^2 ; q = 1/sqrt(v+0.1) ; 0.2R = Sq(sqrt(.2)q)
                nc.scalar.activation(v[:], xy, SQUARE, scale=SQ_SCALE)
                if HILL == "act":
                    nc.scalar.activation(qx[:], v[:, :F], ARSQRT, bias=0.1)
                    nc.scalar.activation(rx02[:], qx[:], SQUARE,
                                         scale=P2_SCALE)
                else:
                    wxf = tp.tile([P, F], f32, tag="wxf", bufs=1)
                    rxf = tp.tile([P, F], f32, tag="rxf", bufs=1)
                    # recip(5*(v+0.1)) = 0.2*Rx
                    ve.tensor_scalar(wxf[:], v[:, :F], 5.0, 0.5, MUL, ADD)
                    ve._custom_dve(RECIPROCAL_APPROX_FAST, out=rxf[:],
                                   in0=wxf[:], **RECIP_APPROX_FAST_CONSTS)
                    ve.tensor_copy(rx02[:], rxf[:])
                nc.scalar.activation(qy[:], v[:, F:], ARSQRT, bias=0.1)
                nc.scalar.activation(aa[:], qy[:], SQUARE, scale=P2_SCALE)
                nc.scalar.activation(ry[:], qy[:], SQUARE)

                # --- combine (DVE, TT 2x + ts 4x) ---
                ve.tensor_scalar(rx10[:], rx02[:], 5.0, 10.0, MUL, SUB)
                ve.tensor_scalar(x11[:], xy[:, :F], 1.1, None, MUL)
                ve.tensor_scalar(y11[:], xy[:, F:], 1.1, 10.0, MUL, SUB)
                ve.tensor_tensor(gg[:], rx10[:], u1[:], MUL)
                ve.tensor_tensor(ca[:], aa[:], x11[:], SUB)
                ve.tensor_tensor(dxy[:, :F], ca[:], gg[:], SUB)
                ve.tensor_tensor(d1[:], rx02[:], y11[:], SUB)
                ve.tensor_tensor(dxy[:, F:], d1[:], ry[:], SUB)

                # ndxy = -dxy via fp16 sign-bit flip (int16 XOR, ts 4x)
                ve.tensor_scalar(ndxy[:].bitcast(i16), dxy[:].bitcast(i16),
                                 -32768, None, XOR)

                nc.sync.dma_start(o_d[:, 4 * F * c:4 * F * c + 2 * F], dxy[:])
                nc.sync.dma_start(o_d[:, 4 * F * c + 2 * F:4 * F * (c + 1)],
                                  ndxy[:])

    nc.compile()
    return nc


def _get_nc(t0: float, t1: float):
    key = (t0, t1, HILL, GP_OFFLOAD, F)
    if key not in _COMPILED:
        _COMPILED[key] = _build(t0, t1)
    return _COMPILED[key]


def run_sharded(x, y, e_x, e_y, W_a, target, trace=False, **run_kwargs):
    """Shard inputs over 8 cores, run the Bass kernel, gather full output.

    Returns (out[B,4] float32, BassKernelResults).
    """
    from concourse.bass_utils import run_bass_kernel_spmd

    x = np.ascontiguousarray(x, dtype=np.float32)
    y = np.ascontiguousarray(y, dtype=np.float32)
    e_x = np.ascontiguousarray(e_x, dtype=np.float32)
    e_y = np.ascontiguousarray(e_y, dtype=np.float32)
    W_a = np.ascontiguousarray(W_a, dtype=np.float32)
    target = np.asarray(target, dtype=np.float32)
    assert x.shape == (B,) and W_a.shape == (B, 2) and target.shape == (2,)

    t0, t1 = float(target[0]), float(target[1])
    nc = _get_nc(t0, t1)

    # Host-side packing (sharding/layout only): per-chunk blocks so each
    # tile is one DMA.  fp16 cast is the device-precision choice.
    pk = np.empty((N_CORES, P, N_IT, 6 * F), dtype=np.float16)
    pk[:, :, :, 0 * F:1 * F] = x.reshape(N_CORES, P, N_IT, F)
    pk[:, :, :, 1 * F:2 * F] = y.reshape(N_CORES, P, N_IT, F)
    pk[:, :, :, 2 * F:3 * F] = e_x.reshape(N_CORES, P, N_IT, F)
    pk[:, :, :, 3 * F:4 * F] = e_y.reshape(N_CORES, P, N_IT, F)
    wv = W_a.reshape(N_CORES, P, N_IT, F, 2)
    pk[:, :, :, 4 * F:5 * F] = wv[..., 0]
    pk[:, :, :, 5 * F:6 * F] = wv[..., 1]
    pk = pk.reshape(N_CORES, P, 6 * COLS)

    in_maps = [{"inp": pk[i]} for i in range(N_CORES)]

    res = run_bass_kernel_spmd(nc, in_maps, list(range(N_CORES)),
                               trace=trace, **run_kwargs)
    # unshard: od[P, 4*COLS] per core; per chunk c the columns are
    # [dx(F) | dy(F) | -dx(F) | -dy(F)]
    out = np.empty((B, 4), dtype=np.float32)
    ob = out.reshape(N_CORES, P, N_IT, F, 4)
    for i in range(N_CORES):
        od = res.results[i]["out"].reshape(P, N_IT, 4, F)
        ob[i] = od.transpose(0, 1, 3, 2).astype(np.float32)
    return out, res


def kernel(x, y, e_x, e_y, W_a, target):
    out, _ = run_sharded(x, y, e_x, e_y, W_a, target)
    return out


# revision 8
# speedup vs baseline: 1.4621x; 1.4621x over previous
"""Trainium2 Bass kernel for the batched CA_event ODE-RHS problem.

Computes, for B = 8388608 independent systems (per batch element):
    u  = W0*(x+e_x-t0) + W1*(y+e_y-t1)
    R_s = 1/(0.004*s^2+0.1)            # 10*(1-hill(s))
    dx = (10-Rx)*(1+u) + 0.2*Ry - 1.1*x
    dy = (10-Ry) + 0.2*Rx - 1.1*y
    out = [dx, dy, -dx, -dy]           # shape [B, 4]

Memory-bound problem; all device I/O is fp16 (harness gate is
scale-relative 2e-2; this pipeline lands ~2.5e-3).  Engine split (the
GpSimd engine is deliberately idle - its SBUF streaming contends with
DVE ports and stretches DVE ops 2-4x):

  ACT: v=Sq(.0632*xy) ; qx,qy=arsqrt(v+.1) ; rx02=Sq(sqrt(.2)qx)
       (=0.2Rx) ; aa=Sq(sqrt(.2)qy) (=0.2Ry) ; ry=Sq(qy)
  DVE: pqa=xy+exy (TT 2x) ; pq=pqa-t (ts 4x) ; m=wt*pq (TT) ;
       s1=m0+m1 (TT) ; u1=s1+1 (ts) ; rx10=5*rx02-10 ; x11=1.1x ;
       y11=1.1y-10 (ts) ; gg=rx10*u1 ; ca=aa-x11 ; dx=ca-gg ;
       d1=rx02-y11 ; dy=d1-ry (TT) ; ndxy=dxy^0x80008000 (int32 ts)

scalar_tensor_tensor runs at 1x only (no fp16 2x uop) so every chain is
built from tensor_tensor (2x) + tensor_scalar (4x) instead.

Outputs are written as planes [dx|dy] and [-dx|-dy] per chunk; the host
restacks to [B, 4] (pure gather, no math).  Batch is split evenly
across 8 NeuronCores; per-core 1048576 elements viewed as [128, 8192].
Chunks are non-uniform (small first/last) to shrink pipeline head/tail.
"""

import sys

import numpy as np

try:
    import concourse  # noqa: F401
except ImportError:  # pragma: no cover - fallback for bare environments
    sys.path.insert(0, "/opt/trn_rl_repo")

B = 8388608
N_CORES = 8
P = 128
BC = B // N_CORES          # 1048576 elements per core
COLS = BC // P             # 8192 free-dim columns per core
F = 2048                   # max tile columns per loop iteration

# non-uniform chunk schedule (must sum to COLS): small head/tail chunks
CHUNKS = [1024, 2048, 2048, 2048, 1024]
assert sum(CHUNKS) == COLS

_COMPILED = {}

# config knobs (overridable from test.py for A/B runs)
FAST_RECIP = False         # kept for test.py compat (unused)
HILL = "act"               # "act": both R via ACT splines; "recip": Rx on DVE

SQ_SCALE = 0.0632455532    # sqrt(0.004): Square(SQ_SCALE*s) = 0.004*s^2
P2_SCALE = 0.4472135955    # sqrt(0.2):   Square(P2_SCALE*q) = 0.2*q^2


def _build(t0: float, t1: float):
    """Trace + compile the per-core Tile kernel. Returns a ready Bass object."""
    from contextlib import ExitStack

    import concourse.bacc as bacc
    import concourse.tile as tile
    from concourse import mybir
    from concourse.dve_ops import (
        RECIP_APPROX_FAST_CONSTS,
        RECIPROCAL_APPROX_FAST,
    )

    f16 = mybir.dt.float16
    f32 = mybir.dt.float32
    i32 = mybir.dt.int32
    ADD = mybir.AluOpType.add
    SUB = mybir.AluOpType.subtract
    MUL = mybir.AluOpType.mult
    XOR = mybir.AluOpType.bitwise_xor
    SQUARE = mybir.ActivationFunctionType.Square
    ARSQRT = mybir.ActivationFunctionType.Abs_reciprocal_sqrt

    assert t0 == t1

    nc = bacc.Bacc("TRN2", target_bir_lowering=False, debug=False,
                   num_devices=N_CORES)

    # bias constant for the arsqrt activation (bias APs must pre-exist)
    _c = nc.alloc_sbuf_tensor("const-float32-0.1", [128, 1], f32)
    nc.gpsimd.memset(_c.ap(), 0.1)
    nc.const_aps.aps[(f32, 0.1)] = _c.ap()
    nc.all_engine_barrier()

    in_d = nc.dram_tensor("inp", [P, 6 * COLS], f16,
                          kind="ExternalInput").ap()
    o_d = nc.dram_tensor("out", [P, 4 * COLS], f16, kind="ExternalOutput").ap()

    ve = nc.vector

    with tile.TileContext(nc) as tc:
        with ExitStack() as ctx:
            io = ctx.enter_context(tc.tile_pool(name="io", bufs=2))
            tp = ctx.enter_context(tc.tile_pool(name="tmp", bufs=2))

            off = 0
            for fsz in CHUNKS:
                # full-width tiles, sliced to fsz (keeps pool geometry fixed)
                itf = io.tile([P, 6 * F], f16, tag="in")
                dxyf = io.tile([P, 2 * F], f16, tag="dxy")
                ndxyf = io.tile([P, 2 * F], f16, tag="ndxy")
                dxy = dxyf[:, :2 * fsz]
                ndxy = ndxyf[:, :2 * fsz]

                # packed layout per chunk: [x|y|ex|ey|W0|W1], fsz cols each
                nc.sync.dma_start(itf[:, :4 * fsz],
                                  in_d[:, 6 * off:6 * off + 4 * fsz])
                nc.sync.dma_start(itf[:, 4 * fsz:6 * fsz],
                                  in_d[:, 6 * off + 4 * fsz:6 * (off + fsz)])
                xy = itf[:, :2 * fsz]
                exy = itf[:, 2 * fsz:4 * fsz]
                wt = itf[:, 4 * fsz:6 * fsz]

                pqa = tp.tile([P, 2 * F], f16, tag="pqa", bufs=1)
                pq = tp.tile([P, 2 * F], f16, tag="pq", bufs=1)
                m = tp.tile([P, 2 * F], f16, tag="m", bufs=1)
                s1 = tp.tile([P, F], f16, tag="s1", bufs=1)
                u1 = tp.tile([P, F], f16, tag="u1", bufs=1)
                v = tp.tile([P, 2 * F], f16, tag="v", bufs=1)
                qx = tp.tile([P, F], f16, tag="qx", bufs=1)
                qy = tp.tile([P, F], f16, tag="qy", bufs=1)
                rx02 = tp.tile([P, F], f16, tag="rx02")
                aa = tp.tile([P, F], f16, tag="aa")
                ry = tp.tile([P, F], f16, tag="ry")
                rx10 = tp.tile([P, F], f16, tag="rx10", bufs=1)
                x11 = tp.tile([P, F], f16, tag="x11", bufs=1)
                y11 = tp.tile([P, F], f16, tag="y11", bufs=1)
                gg = tp.tile([P, F], f16, tag="gg", bufs=1)
                ca = tp.tile([P, F], f16, tag="ca", bufs=1)
                d1 = tp.tile([P, F], f16, tag="d1", bufs=1)

                # --- u path (DVE) ---
                # pq = (xy+exy) - t ; m = wt*pq ; u1 = (m0+m1) + 1
                ve.tensor_tensor(pqa[:, :2 * fsz], xy, exy, ADD)
                ve.tensor_scalar(pq[:, :2 * fsz], pqa[:, :2 * fsz], 1.0, t0,
                                 MUL, SUB)
                ve.tensor_tensor(m[:, :2 * fsz], wt, pq[:, :2 * fsz], MUL)
                ve.tensor_tensor(s1[:, :fsz], m[:, :fsz], m[:, fsz:2 * fsz],
                                 ADD)
                ve.tensor_scalar(u1[:, :fsz], s1[:, :fsz], 1.0, None, ADD)

                # --- Hill path (ACT) ---
                # v = 0.004*s^2 ; q = 1/sqrt(v+0.1) ; 0.2R = Sq(sqrt(.2)q)
                nc.scalar.activation(v[:, :2 * fsz], xy, SQUARE,
                                     scale=SQ_SCALE)
                if HILL == "act":
                    nc.scalar.activation(qx[:, :fsz], v[:, :fsz], ARSQRT,
                                         bias=0.1)
                    nc.scalar.activation(rx02[:, :fsz], qx[:, :fsz], SQUARE,
                                         scale=P2_SCALE)
                else:
                    wxf = tp.tile([P, F], f32, tag="wxf", bufs=1)
                    rxf = tp.tile([P, F], f32, tag="rxf", bufs=1)
                    # recip(5*(v+0.1)) = 0.2*Rx
                    ve.tensor_scalar(wxf[:, :fsz], v[:, :fsz], 5.0, 0.5,
                                     MUL, ADD)
                    ve._custom_dve(RECIPROCAL_APPROX_FAST,
                                   out=rxf[:, :fsz], in0=wxf[:, :fsz],
                                   **RECIP_APPROX_FAST_CONSTS)
                    ve.tensor_copy(rx02[:, :fsz], rxf[:, :fsz])
                nc.scalar.activation(qy[:, :fsz], v[:, fsz:2 * fsz], ARSQRT,
                                     bias=0.1)
                nc.scalar.activation(aa[:, :fsz], qy[:, :fsz], SQUARE,
                                     scale=P2_SCALE)
                nc.scalar.activation(ry[:, :fsz], qy[:, :fsz], SQUARE)

                # --- combine (DVE, TT 2x + ts 4x) ---
                ve.tensor_scalar(rx10[:, :fsz], rx02[:, :fsz], 5.0, 10.0,
                                 MUL, SUB)
                ve.tensor_scalar(x11[:, :fsz], xy[:, :fsz], 1.1, None, MUL)
                ve.tensor_scalar(y11[:, :fsz], xy[:, fsz:2 * fsz], 1.1, 10.0,
                                 MUL, SUB)
                ve.tensor_tensor(gg[:, :fsz], rx10[:, :fsz], u1[:, :fsz], MUL)
                ve.tensor_tensor(ca[:, :fsz], aa[:, :fsz], x11[:, :fsz], SUB)
                ve.tensor_tensor(dxy[:, :fsz], ca[:, :fsz], gg[:, :fsz], SUB)
                ve.tensor_tensor(d1[:, :fsz], rx02[:, :fsz], y11[:, :fsz],
                                 SUB)
                ve.tensor_tensor(dxy[:, fsz:], d1[:, :fsz], ry[:, :fsz], SUB)

                # ndxy = -dxy: fp16 sign flip, two-at-a-time as int32 XOR
                ve.tensor_scalar(ndxy.bitcast(i32), dxy.bitcast(i32),
                                 -2147450880, None, XOR)

                nc.sync.dma_start(o_d[:, 4 * off:4 * off + 2 * fsz], dxy)
                nc.sync.dma_start(o_d[:, 4 * off + 2 * fsz:4 * (off + fsz)],
                                  ndxy)
                off += fsz

    nc.compile()
    return nc


def _get_nc(t0: float, t1: float):
    key = (t0, t1, HILL, tuple(CHUNKS))
    if key not in _COMPILED:
        _COMPILED[key] = _build(t0, t1)
    return _COMPILED[key]


def run_sharded(x, y, e_x, e_y, W_a, target, trace=False, **run_kwargs):
    """Shard inputs over 8 cores, run the Bass kernel, gather full output.

    Returns (out[B,4] float32, BassKernelResults).
    """
    from concourse.bass_utils import run_bass_kernel_spmd

    x = np.ascontiguousarray(x, dtype=np.float32)
    y = np.ascontiguousarray(y, dtype=np.float32)
    e_x = np.ascontiguousarray(e_x, dtype=np.float32)
    e_y = np.ascontiguousarray(e_y, dtype=np.float32)
    W_a = np.ascontiguousarray(W_a, dtype=np.float32)
    target = np.asarray(target, dtype=np.float32)
    assert x.shape == (B,) and W_a.shape == (B, 2) and target.shape == (2,)

    t0, t1 = float(target[0]), float(target[1])
    nc = _get_nc(t0, t1)

    # Host-side packing (sharding/layout only): per-chunk blocks of
    # [x|y|ex|ey|W0|W1], fsz cols each.  fp16 cast is the device-precision
    # choice.
    xs = x.reshape(N_CORES, P, COLS)
    ys = y.reshape(N_CORES, P, COLS)
    exs = e_x.reshape(N_CORES, P, COLS)
    eys = e_y.reshape(N_CORES, P, COLS)
    w0 = W_a[:, 0].reshape(N_CORES, P, COLS)
    w1 = W_a[:, 1].reshape(N_CORES, P, COLS)
    pk = np.empty((N_CORES, P, 6 * COLS), dtype=np.float16)
    off = 0
    for fsz in CHUNKS:
        base = 6 * off
        sl = slice(off, off + fsz)
        pk[:, :, base + 0 * fsz:base + 1 * fsz] = xs[:, :, sl]
        pk[:, :, base + 1 * fsz:base + 2 * fsz] = ys[:, :, sl]
        pk[:, :, base + 2 * fsz:base + 3 * fsz] = exs[:, :, sl]
        pk[:, :, base + 3 * fsz:base + 4 * fsz] = eys[:, :, sl]
        pk[:, :, base + 4 * fsz:base + 5 * fsz] = w0[:, :, sl]
        pk[:, :, base + 5 * fsz:base + 6 * fsz] = w1[:, :, sl]
        off += fsz

    in_maps = [{"inp": pk[i]} for i in range(N_CORES)]

    res = run_bass_kernel_spmd(nc, in_maps, list(range(N_CORES)),
                               trace=trace, **run_kwargs)
    # unshard: od[P, 4*COLS] per core; per chunk the columns are
    # [dx(fsz) | dy(fsz) | -dx(fsz) | -dy(fsz)]
    out = np.empty((B, 4), dtype=np.float32)
    ob = out.reshape(N_CORES, P, COLS, 4)
    for i in range(N_CORES):
        od = res.results[i]["out"]
        off = 0
        for fsz in CHUNKS:
            blk = od[:, 4 * off:4 * (off + fsz)].reshape(P, 4, fsz)
            ob[i, :, off:off + fsz] = blk.transpose(0, 2, 1)
            off += fsz
    return out, res


def kernel(x, y, e_x, e_y, W_a, target):
    out, _ = run_sharded(x, y, e_x, e_y, W_a, target)
    return out


# revision 13
# speedup vs baseline: 1.4695x; 1.0051x over previous
"""Trainium2 Bass kernel for the batched CA_event ODE-RHS problem.

Computes, for B = 8388608 independent systems (per batch element):
    u  = W0*(x+e_x-t0) + W1*(y+e_y-t1)
    R_s = 1/(0.004*s^2+0.1)            # 10*(1-hill(s))
    dx = (10-Rx)*(1+u) + 0.2*Ry - 1.1*x
    dy = (10-Ry) + 0.2*Rx - 1.1*y
    out = [dx, dy, -dx, -dy]           # shape [B, 4]

Memory-bound problem; all device I/O is fp16 (harness gate is
scale-relative 2e-2; this pipeline lands ~2.5e-3).  Engine split (the
GpSimd engine is deliberately idle - its SBUF streaming contends with
DVE ports and stretches DVE ops 2-4x):

  ACT: v=Sq(.0632*xy) ; qx,qy=arsqrt(v+.1) ; rx02=Sq(sqrt(.2)qx)
       (=0.2Rx) ; aa=Sq(sqrt(.2)qy) (=0.2Ry) ; ry=Sq(qy)
  DVE: pqa=xy+exy (TT 2x) ; pq=pqa-t (ts 4x) ; m=wt*pq (TT) ;
       s1=m0+m1 (TT) ; u1=s1+1 (ts) ; rx10=5*rx02-10 ; x11=1.1x ;
       y11=1.1y-10 (ts) ; gg=rx10*u1 ; ca=aa-x11 ; dx=ca-gg ;
       d1=rx02-y11 ; dy=d1-ry (TT) ; ndxy=dxy^0x80008000 (int32 ts)

scalar_tensor_tensor runs at 1x only (no fp16 2x uop) so every chain is
built from tensor_tensor (2x) + tensor_scalar (4x) instead.

Outputs are written as planes [dx|dy] and [-dx|-dy] per chunk; the host
restacks to [B, 4] (pure gather, no math).  Batch is split evenly
across 8 NeuronCores; per-core 1048576 elements viewed as [128, 8192].
Chunks are non-uniform (small first/last) to shrink pipeline head/tail.
"""

import sys

import numpy as np

try:
    import concourse  # noqa: F401
except ImportError:  # pragma: no cover - fallback for bare environments
    sys.path.insert(0, "/opt/trn_rl_repo")

B = 8388608
N_CORES = 8
P = 128
BC = B // N_CORES          # 1048576 elements per core
COLS = BC // P             # 8192 free-dim columns per core
F = 2048                   # max tile columns per loop iteration

# non-uniform chunk schedule (must sum to COLS): small head/tail chunks
CHUNKS = [512, 1536, 2048, 2048, 1536, 512]
assert sum(CHUNKS) == COLS

_COMPILED = {}

# config knobs (overridable from test.py for A/B runs)
FAST_RECIP = False         # kept for test.py compat (unused)
HILL = "act"               # "act": both R via ACT splines; "recip": Rx on DVE

SQ_SCALE = 0.0632455532    # sqrt(0.004): Square(SQ_SCALE*s) = 0.004*s^2
P2_SCALE = 0.4472135955    # sqrt(0.2):   Square(P2_SCALE*q) = 0.2*q^2


def _build(t0: float, t1: float):
    """Trace + compile the per-core Tile kernel. Returns a ready Bass object."""
    from contextlib import ExitStack

    import concourse.bacc as bacc
    import concourse.tile as tile
    from concourse import mybir
    from concourse.dve_ops import (
        RECIP_APPROX_FAST_CONSTS,
        RECIPROCAL_APPROX_FAST,
    )

    f16 = mybir.dt.float16
    f32 = mybir.dt.float32
    i32 = mybir.dt.int32
    ADD = mybir.AluOpType.add
    SUB = mybir.AluOpType.subtract
    MUL = mybir.AluOpType.mult
    XOR = mybir.AluOpType.bitwise_xor
    SQUARE = mybir.ActivationFunctionType.Square
    ARSQRT = mybir.ActivationFunctionType.Abs_reciprocal_sqrt

    assert t0 == t1

    nc = bacc.Bacc("TRN2", target_bir_lowering=False, debug=False,
                   num_devices=N_CORES)

    # bias constant for the arsqrt activation (bias APs must pre-exist)
    _c = nc.alloc_sbuf_tensor("const-float32-0.1", [128, 1], f32)
    nc.gpsimd.memset(_c.ap(), 0.1)
    nc.const_aps.aps[(f32, 0.1)] = _c.ap()
    nc.all_engine_barrier()

    in_d = nc.dram_tensor("inp", [P, 6 * COLS], f16,
                          kind="ExternalInput").ap()
    o_d = nc.dram_tensor("out", [P, 4 * COLS], f16, kind="ExternalOutput").ap()

    ve = nc.vector

    with tile.TileContext(nc) as tc:
        with ExitStack() as ctx:
            io = ctx.enter_context(tc.tile_pool(name="io", bufs=2))
            tp = ctx.enter_context(tc.tile_pool(name="tmp", bufs=2))

            off = 0
            for fsz in CHUNKS:
                # full-width tiles, sliced to fsz (keeps pool geometry fixed)
                itf = io.tile([P, 6 * F], f16, tag="in")
                dxyf = io.tile([P, 2 * F], f16, tag="dxy")
                ndxyf = io.tile([P, 2 * F], f16, tag="ndxy")
                dxy = dxyf[:, :2 * fsz]
                ndxy = ndxyf[:, :2 * fsz]

                # packed layout per chunk: [x|y|ex|ey|W0|W1], fsz cols each.
                # xy lands first so ACT's v-chain starts earliest.
                nc.sync.dma_start(itf[:, :2 * fsz],
                                  in_d[:, 6 * off:6 * off + 2 * fsz])
                nc.sync.dma_start(itf[:, 2 * fsz:4 * fsz],
                                  in_d[:, 6 * off + 2 * fsz:6 * off + 4 * fsz])
                nc.sync.dma_start(itf[:, 4 * fsz:6 * fsz],
                                  in_d[:, 6 * off + 4 * fsz:6 * (off + fsz)])
                xy = itf[:, :2 * fsz]
                exy = itf[:, 2 * fsz:4 * fsz]
                wt = itf[:, 4 * fsz:6 * fsz]

                pqa = tp.tile([P, 2 * F], f16, tag="pqa", bufs=1)
                pq = tp.tile([P, 2 * F], f16, tag="pq", bufs=1)
                m = tp.tile([P, 2 * F], f16, tag="m", bufs=1)
                s1 = tp.tile([P, F], f16, tag="s1", bufs=1)
                u1 = tp.tile([P, F], f16, tag="u1", bufs=1)
                v = tp.tile([P, 2 * F], f16, tag="v")
                qx = tp.tile([P, F], f16, tag="qx", bufs=1)
                qy = tp.tile([P, F], f16, tag="qy", bufs=1)
                rx02 = tp.tile([P, F], f16, tag="rx02")
                aa = tp.tile([P, F], f16, tag="aa")
                ry = tp.tile([P, F], f16, tag="ry")
                rx10 = tp.tile([P, F], f16, tag="rx10", bufs=1)
                x11 = tp.tile([P, F], f16, tag="x11", bufs=1)
                y11 = tp.tile([P, F], f16, tag="y11", bufs=1)
                gg = tp.tile([P, F], f16, tag="gg", bufs=1)
                ca = tp.tile([P, F], f16, tag="ca", bufs=1)
                d1 = tp.tile([P, F], f16, tag="d1", bufs=1)

                # --- u path + input-tile consumers (DVE, ACT-independent;
                # emitted first so `it` frees early and DVE rides out the
                # ACT chain latency) ---
                # pq = (xy+exy) - t ; m = wt*pq ; u1 = (m0+m1) + 1
                ve.tensor_tensor(pqa[:, :2 * fsz], xy, exy, ADD)
                ve.tensor_scalar(pq[:, :2 * fsz], pqa[:, :2 * fsz], 1.0, t0,
                                 MUL, SUB)
                ve.tensor_scalar(x11[:, :fsz], xy[:, :fsz], 1.1, None, MUL)
                ve.tensor_scalar(y11[:, :fsz], xy[:, fsz:2 * fsz], 1.1, 10.0,
                                 MUL, SUB)
                ve.tensor_tensor(m[:, :2 * fsz], wt, pq[:, :2 * fsz], MUL)
                ve.tensor_tensor(s1[:, :fsz], m[:, :fsz], m[:, fsz:2 * fsz],
                                 ADD)
                ve.tensor_scalar(u1[:, :fsz], s1[:, :fsz], 1.0, None, ADD)

                # --- Hill path (ACT) ---
                # v = 0.004*s^2 ; q = 1/sqrt(v+0.1) ; 0.2R = Sq(sqrt(.2)q)
                nc.scalar.activation(v[:, :2 * fsz], xy, SQUARE,
                                     scale=SQ_SCALE)
                if HILL == "act":
                    nc.scalar.activation(qx[:, :fsz], v[:, :fsz], ARSQRT,
                                         bias=0.1)
                    nc.scalar.activation(rx02[:, :fsz], qx[:, :fsz], SQUARE,
                                         scale=P2_SCALE)
                else:
                    wxf = tp.tile([P, F], f32, tag="wxf", bufs=1)
                    rxf = tp.tile([P, F], f32, tag="rxf", bufs=1)
                    # recip(5*(v+0.1)) = 0.2*Rx
                    ve.tensor_scalar(wxf[:, :fsz], v[:, :fsz], 5.0, 0.5,
                                     MUL, ADD)
                    ve._custom_dve(RECIPROCAL_APPROX_FAST,
                                   out=rxf[:, :fsz], in0=wxf[:, :fsz],
                                   **RECIP_APPROX_FAST_CONSTS)
                    ve.tensor_copy(rx02[:, :fsz], rxf[:, :fsz])
                nc.scalar.activation(qy[:, :fsz], v[:, fsz:2 * fsz], ARSQRT,
                                     bias=0.1)
                nc.scalar.activation(aa[:, :fsz], qy[:, :fsz], SQUARE,
                                     scale=P2_SCALE)
                nc.scalar.activation(ry[:, :fsz], qy[:, :fsz], SQUARE)

                # --- combine (DVE, TT 2x + ts 4x) ---
                ve.tensor_scalar(rx10[:, :fsz], rx02[:, :fsz], 5.0, 10.0,
                                 MUL, SUB)
                ve.tensor_tensor(gg[:, :fsz], rx10[:, :fsz], u1[:, :fsz], MUL)
                ve.tensor_tensor(ca[:, :fsz], aa[:, :fsz], x11[:, :fsz], SUB)
                ve.tensor_tensor(dxy[:, :fsz], ca[:, :fsz], gg[:, :fsz], SUB)
                ve.tensor_tensor(d1[:, :fsz], rx02[:, :fsz], y11[:, :fsz],
                                 SUB)
                ve.tensor_tensor(dxy[:, fsz:], d1[:, :fsz], ry[:, :fsz], SUB)

                # ndxy = -dxy: fp16 sign flip, two-at-a-time as int32 XOR
                ve.tensor_scalar(ndxy.bitcast(i32), dxy.bitcast(i32),
                                 -2147450880, None, XOR)

                nc.sync.dma_start(o_d[:, 4 * off:4 * off + 2 * fsz], dxy)
                nc.sync.dma_start(o_d[:, 4 * off + 2 * fsz:4 * (off + fsz)],
                                  ndxy)
                off += fsz

    nc.compile()
    return nc


def _get_nc(t0: float, t1: float):
    key = (t0, t1, HILL, tuple(CHUNKS))
    if key not in _COMPILED:
        _COMPILED[key] = _build(t0, t1)
    return _COMPILED[key]


def run_sharded(x, y, e_x, e_y, W_a, target, trace=False, **run_kwargs):
    """Shard inputs over 8 cores, run the Bass kernel, gather full output.

    Returns (out[B,4] float32, BassKernelResults).
    """
    from concourse.bass_utils import run_bass_kernel_spmd

    x = np.ascontiguousarray(x, dtype=np.float32)
    y = np.ascontiguousarray(y, dtype=np.float32)
    e_x = np.ascontiguousarray(e_x, dtype=np.float32)
    e_y = np.ascontiguousarray(e_y, dtype=np.float32)
    W_a = np.ascontiguousarray(W_a, dtype=np.float32)
    target = np.asarray(target, dtype=np.float32)
    assert x.shape == (B,) and W_a.shape == (B, 2) and target.shape == (2,)

    t0, t1 = float(target[0]), float(target[1])
    nc = _get_nc(t0, t1)

    # Host-side packing (sharding/layout only): per-chunk blocks of
    # [x|y|ex|ey|W0|W1], fsz cols each.  fp16 cast is the device-precision
    # choice.
    xs = x.reshape(N_CORES, P, COLS)
    ys = y.reshape(N_CORES, P, COLS)
    exs = e_x.reshape(N_CORES, P, COLS)
    eys = e_y.reshape(N_CORES, P, COLS)
    w0 = W_a[:, 0].reshape(N_CORES, P, COLS)
    w1 = W_a[:, 1].reshape(N_CORES, P, COLS)
    pk = np.empty((N_CORES, P, 6 * COLS), dtype=np.float16)
    off = 0
    for fsz in CHUNKS:
        base = 6 * off
        sl = slice(off, off + fsz)
        pk[:, :, base + 0 * fsz:base + 1 * fsz] = xs[:, :, sl]
        pk[:, :, base + 1 * fsz:base + 2 * fsz] = ys[:, :, sl]
        pk[:, :, base + 2 * fsz:base + 3 * fsz] = exs[:, :, sl]
        pk[:, :, base + 3 * fsz:base + 4 * fsz] = eys[:, :, sl]
        pk[:, :, base + 4 * fsz:base + 5 * fsz] = w0[:, :, sl]
        pk[:, :, base + 5 * fsz:base + 6 * fsz] = w1[:, :, sl]
        off += fsz

    in_maps = [{"inp": pk[i]} for i in range(N_CORES)]

    res = run_bass_kernel_spmd(nc, in_maps, list(range(N_CORES)),
                               trace=trace, **run_kwargs)
    # unshard: od[P, 4*COLS] per core; per chunk the columns are
    # [dx(fsz) | dy(fsz) | -dx(fsz) | -dy(fsz)]
    out = np.empty((B, 4), dtype=np.float32)
    ob = out.reshape(N_CORES, P, COLS, 4)
    for i in range(N_CORES):
        od = res.results[i]["out"]
        off = 0
        for fsz in CHUNKS:
            blk = od[:, 4 * off:4 * (off + fsz)].reshape(P, 4, fsz)
            ob[i, :, off:off + fsz] = blk.transpose(0, 2, 1)
            off += fsz
    return out, res


def kernel(x, y, e_x, e_y, W_a, target):
    out, _ = run_sharded(x, y, e_x, e_y, W_a, target)
    return out


# revision 16
# speedup vs baseline: 1.4816x; 1.0082x over previous
"""Trainium2 Bass kernel for the batched CA_event ODE-RHS problem.

Computes, for B = 8388608 independent systems (per batch element):
    u  = W0*(x+e_x-t0) + W1*(y+e_y-t1)
    R_s = 1/(0.004*s^2+0.1)            # 10*(1-hill(s))
    dx = (10-Rx)*(1+u) + 0.2*Ry - 1.1*x
    dy = (10-Ry) + 0.2*Rx - 1.1*y
    out = [dx, dy, -dx, -dy]           # shape [B, 4]

Memory-bound problem; all device I/O is fp16 (harness gate is
scale-relative 2e-2; this pipeline lands ~2.5e-3).  Engine split (the
GpSimd engine is deliberately idle - its SBUF streaming contends with
DVE ports and stretches DVE ops 2-4x):

  ACT: v=Sq(.0632*xy) ; qx,qy=arsqrt(v+.1) ; rx02=Sq(sqrt(.2)qx)
       (=0.2Rx) ; aa=Sq(sqrt(.2)qy) (=0.2Ry) ; ry=Sq(qy)
  DVE: pqa=xy+exy (TT 2x) ; pq=pqa-t (ts 4x) ; m=wt*pq (TT) ;
       s1=m0+m1 (TT) ; u1=s1+1 (ts) ; rx10=5*rx02-10 ; x11=1.1x ;
       y11=1.1y-10 (ts) ; gg=rx10*u1 ; ca=aa-x11 ; dx=ca-gg ;
       d1=rx02-y11 ; dy=d1-ry (TT) ; ndxy=dxy^0x80008000 (int32 ts)

scalar_tensor_tensor runs at 1x only (no fp16 2x uop) so every chain is
built from tensor_tensor (2x) + tensor_scalar (4x) instead.

Outputs are written as planes [dx|dy] and [-dx|-dy] per chunk; the host
restacks to [B, 4] (pure gather, no math).  Batch is split evenly
across 8 NeuronCores; per-core 1048576 elements viewed as [128, 8192].
Chunks are non-uniform (small first/last) to shrink pipeline head/tail.
"""

import sys

import numpy as np

try:
    import concourse  # noqa: F401
except ImportError:  # pragma: no cover - fallback for bare environments
    sys.path.insert(0, "/opt/trn_rl_repo")

B = 8388608
N_CORES = 8
P = 128
BC = B // N_CORES          # 1048576 elements per core
COLS = BC // P             # 8192 free-dim columns per core
F = 2048                   # max tile columns per loop iteration

# non-uniform chunk schedule (must sum to COLS): tiny first chunk so
# compute starts early, decreasing tail chunks so the last outputs are
# small and the out-DMA drain after the final compute is short
CHUNKS = [256, 1280, 2048, 2048, 1536, 768, 256]
assert sum(CHUNKS) == COLS

_COMPILED = {}

# config knobs (overridable from test.py for A/B runs)
FAST_RECIP = False         # kept for test.py compat (unused)
HILL = "act"               # "act": both R via ACT splines; "recip": Rx on DVE

SQ_SCALE = 0.0632455532    # sqrt(0.004): Square(SQ_SCALE*s) = 0.004*s^2
P2_SCALE = 0.4472135955    # sqrt(0.2):   Square(P2_SCALE*q) = 0.2*q^2


def _build(t0: float, t1: float):
    """Trace + compile the per-core Tile kernel. Returns a ready Bass object."""
    from contextlib import ExitStack

    import concourse.bacc as bacc
    import concourse.tile as tile
    from concourse import mybir
    from concourse.dve_ops import (
        RECIP_APPROX_FAST_CONSTS,
        RECIPROCAL_APPROX_FAST,
    )

    f16 = mybir.dt.float16
    f32 = mybir.dt.float32
    i32 = mybir.dt.int32
    ADD = mybir.AluOpType.add
    SUB = mybir.AluOpType.subtract
    MUL = mybir.AluOpType.mult
    XOR = mybir.AluOpType.bitwise_xor
    SQUARE = mybir.ActivationFunctionType.Square
    ARSQRT = mybir.ActivationFunctionType.Abs_reciprocal_sqrt
    COPY = mybir.ActivationFunctionType.Copy

    assert t0 == t1

    nc = bacc.Bacc("TRN2", target_bir_lowering=False, debug=False,
                   num_devices=N_CORES)

    # bias constant for the arsqrt activation (bias APs must pre-exist)
    _c = nc.alloc_sbuf_tensor("const-float32-0.1", [128, 1], f32)
    nc.gpsimd.memset(_c.ap(), 0.1)
    nc.const_aps.aps[(f32, 0.1)] = _c.ap()
    nc.all_engine_barrier()

    in_d = nc.dram_tensor("inp", [P, 6 * COLS], f16,
                          kind="ExternalInput").ap()
    o_d = nc.dram_tensor("out", [P, 4 * COLS], f16, kind="ExternalOutput").ap()

    ve = nc.vector

    with tile.TileContext(nc) as tc:
        with ExitStack() as ctx:
            io = ctx.enter_context(tc.tile_pool(name="io", bufs=2))
            tp = ctx.enter_context(tc.tile_pool(name="tmp", bufs=2))

            off = 0
            for fsz in CHUNKS:
                # full-width tiles, sliced to fsz (keeps pool geometry fixed)
                itf = io.tile([P, 6 * F], f16, tag="in")
                dxyf = io.tile([P, 2 * F], f16, tag="dxy")
                ndxyf = io.tile([P, 2 * F], f16, tag="ndxy")
                dxy = dxyf[:, :2 * fsz]
                ndxy = ndxyf[:, :2 * fsz]

                # packed layout per chunk: [x|y|ex|ey|W0|W1], fsz cols each.
                # xy lands first so ACT's v-chain starts earliest.
                nc.sync.dma_start(itf[:, :2 * fsz],
                                  in_d[:, 6 * off:6 * off + 2 * fsz])
                nc.sync.dma_start(itf[:, 2 * fsz:4 * fsz],
                                  in_d[:, 6 * off + 2 * fsz:6 * off + 4 * fsz])
                nc.sync.dma_start(itf[:, 4 * fsz:6 * fsz],
                                  in_d[:, 6 * off + 4 * fsz:6 * (off + fsz)])
                xy = itf[:, :2 * fsz]
                exy = itf[:, 2 * fsz:4 * fsz]
                wt = itf[:, 4 * fsz:6 * fsz]

                pqa = tp.tile([P, 2 * F], f16, tag="pqa", bufs=1)
                pq = tp.tile([P, 2 * F], f16, tag="pq", bufs=1)
                m = tp.tile([P, 2 * F], f16, tag="m", bufs=1)
                s1 = tp.tile([P, F], f16, tag="s1", bufs=1)
                u1 = tp.tile([P, F], f16, tag="u1", bufs=1)
                v = tp.tile([P, 2 * F], f16, tag="v")
                qx = tp.tile([P, F], f16, tag="qx", bufs=1)
                qy = tp.tile([P, F], f16, tag="qy", bufs=1)
                rx02 = tp.tile([P, F], f16, tag="rx02")
                aa = tp.tile([P, F], f16, tag="aa")
                ry = tp.tile([P, F], f16, tag="ry")
                rx10 = tp.tile([P, F], f16, tag="rx10", bufs=1)
                x11 = tp.tile([P, F], f16, tag="x11", bufs=1)
                y11 = tp.tile([P, F], f16, tag="y11", bufs=1)
                gg = tp.tile([P, F], f16, tag="gg", bufs=1)
                ca = tp.tile([P, F], f16, tag="ca", bufs=1)
                d1 = tp.tile([P, F], f16, tag="d1", bufs=1)

                # --- u path + input-tile consumers (DVE, ACT-independent;
                # emitted first so `it` frees early and DVE rides out the
                # ACT chain latency) ---
                # pq = (xy+exy) - t ; m = wt*pq ; u1 = (m0+m1) + 1
                ve.tensor_tensor(pqa[:, :2 * fsz], xy, exy, ADD)
                ve.tensor_scalar(pq[:, :2 * fsz], pqa[:, :2 * fsz], 1.0, t0,
                                 MUL, SUB)
                ve.tensor_scalar(x11[:, :fsz], xy[:, :fsz], 1.1, None, MUL)
                # y11 = 1.1y-10 on ACT (Copy allows float bias); ACT has
                # headroom vs DVE
                nc.scalar.activation(y11[:, :fsz], xy[:, fsz:2 * fsz], COPY,
                                     scale=1.1, bias=-10.0)
                ve.tensor_tensor(m[:, :2 * fsz], wt, pq[:, :2 * fsz], MUL)
                ve.tensor_tensor(s1[:, :fsz], m[:, :fsz], m[:, fsz:2 * fsz],
                                 ADD)
                ve.tensor_scalar(u1[:, :fsz], s1[:, :fsz], 1.0, None, ADD)

                # --- Hill path (ACT) ---
                # v = 0.004*s^2 ; q = 1/sqrt(v+0.1) ; 0.2R = Sq(sqrt(.2)q)
                nc.scalar.activation(v[:, :2 * fsz], xy, SQUARE,
                                     scale=SQ_SCALE)
                if HILL == "act":
                    nc.scalar.activation(qx[:, :fsz], v[:, :fsz], ARSQRT,
                                         bias=0.1)
                    nc.scalar.activation(rx02[:, :fsz], qx[:, :fsz], SQUARE,
                                         scale=P2_SCALE)
                else:
                    wxf = tp.tile([P, F], f32, tag="wxf", bufs=1)
                    rxf = tp.tile([P, F], f32, tag="rxf", bufs=1)
                    # recip(5*(v+0.1)) = 0.2*Rx
                    ve.tensor_scalar(wxf[:, :fsz], v[:, :fsz], 5.0, 0.5,
                                     MUL, ADD)
                    ve._custom_dve(RECIPROCAL_APPROX_FAST,
                                   out=rxf[:, :fsz], in0=wxf[:, :fsz],
                                   **RECIP_APPROX_FAST_CONSTS)
                    ve.tensor_copy(rx02[:, :fsz], rxf[:, :fsz])
                nc.scalar.activation(qy[:, :fsz], v[:, fsz:2 * fsz], ARSQRT,
                                     bias=0.1)
                nc.scalar.activation(aa[:, :fsz], qy[:, :fsz], SQUARE,
                                     scale=P2_SCALE)
                nc.scalar.activation(ry[:, :fsz], qy[:, :fsz], SQUARE)

                # --- combine (DVE, TT 2x + ts 4x) ---
                ve.tensor_scalar(rx10[:, :fsz], rx02[:, :fsz], 5.0, 10.0,
                                 MUL, SUB)
                ve.tensor_tensor(gg[:, :fsz], rx10[:, :fsz], u1[:, :fsz], MUL)
                ve.tensor_tensor(ca[:, :fsz], aa[:, :fsz], x11[:, :fsz], SUB)
                ve.tensor_tensor(dxy[:, :fsz], ca[:, :fsz], gg[:, :fsz], SUB)
                ve.tensor_tensor(d1[:, :fsz], rx02[:, :fsz], y11[:, :fsz],
                                 SUB)
                ve.tensor_tensor(dxy[:, fsz:], d1[:, :fsz], ry[:, :fsz], SUB)

                # ndxy = -dxy: fp16 sign flip, two-at-a-time as int32 XOR
                ve.tensor_scalar(ndxy.bitcast(i32), dxy.bitcast(i32),
                                 -2147450880, None, XOR)

                nc.sync.dma_start(o_d[:, 4 * off:4 * off + 2 * fsz], dxy)
                nc.sync.dma_start(o_d[:, 4 * off + 2 * fsz:4 * (off + fsz)],
                                  ndxy)
                off += fsz

    nc.compile()
    return nc


def _get_nc(t0: float, t1: float):
    key = (t0, t1, HILL, tuple(CHUNKS))
    if key not in _COMPILED:
        _COMPILED[key] = _build(t0, t1)
    return _COMPILED[key]


def run_sharded(x, y, e_x, e_y, W_a, target, trace=False, **run_kwargs):
    """Shard inputs over 8 cores, run the Bass kernel, gather full output.

    Returns (out[B,4] float32, BassKernelResults).
    """
    from concourse.bass_utils import run_bass_kernel_spmd

    x = np.ascontiguousarray(x, dtype=np.float32)
    y = np.ascontiguousarray(y, dtype=np.float32)
    e_x = np.ascontiguousarray(e_x, dtype=np.float32)
    e_y = np.ascontiguousarray(e_y, dtype=np.float32)
    W_a = np.ascontiguousarray(W_a, dtype=np.float32)
    target = np.asarray(target, dtype=np.float32)
    assert x.shape == (B,) and W_a.shape == (B, 2) and target.shape == (2,)

    t0, t1 = float(target[0]), float(target[1])
    nc = _get_nc(t0, t1)

    # Host-side packing (sharding/layout only): per-chunk blocks of
    # [x|y|ex|ey|W0|W1], fsz cols each.  fp16 cast is the device-precision
    # choice.
    xs = x.reshape(N_CORES, P, COLS)
    ys = y.reshape(N_CORES, P, COLS)
    exs = e_x.reshape(N_CORES, P, COLS)
    eys = e_y.reshape(N_CORES, P, COLS)
    w0 = W_a[:, 0].reshape(N_CORES, P, COLS)
    w1 = W_a[:, 1].reshape(N_CORES, P, COLS)
    pk = np.empty((N_CORES, P, 6 * COLS), dtype=np.float16)
    off = 0
    for fsz in CHUNKS:
        base = 6 * off
        sl = slice(off, off + fsz)
        pk[:, :, base + 0 * fsz:base + 1 * fsz] = xs[:, :, sl]
        pk[:, :, base + 1 * fsz:base + 2 * fsz] = ys[:, :, sl]
        pk[:, :, base + 2 * fsz:base + 3 * fsz] = exs[:, :, sl]
        pk[:, :, base + 3 * fsz:base + 4 * fsz] = eys[:, :, sl]
        pk[:, :, base + 4 * fsz:base + 5 * fsz] = w0[:, :, sl]
        pk[:, :, base + 5 * fsz:base + 6 * fsz] = w1[:, :, sl]
        off += fsz

    in_maps = [{"inp": pk[i]} for i in range(N_CORES)]

    res = run_bass_kernel_spmd(nc, in_maps, list(range(N_CORES)),
                               trace=trace, **run_kwargs)
    # unshard: od[P, 4*COLS] per core; per chunk the columns are
    # [dx(fsz) | dy(fsz) | -dx(fsz) | -dy(fsz)]
    out = np.empty((B, 4), dtype=np.float32)
    ob = out.reshape(N_CORES, P, COLS, 4)
    for i in range(N_CORES):
        od = res.results[i]["out"]
        off = 0
        for fsz in CHUNKS:
            blk = od[:, 4 * off:4 * (off + fsz)].reshape(P, 4, fsz)
            ob[i, :, off:off + fsz] = blk.transpose(0, 2, 1)
            off += fsz
    return out, res


def kernel(x, y, e_x, e_y, W_a, target):
    out, _ = run_sharded(x, y, e_x, e_y, W_a, target)
    return out
